# revision 1
# baseline (speedup 1.0000x reference)
"""GNN edge-MLP classifier kernel for 8 Trainium2 NeuronCores.

Reference computation (per edge e):
    x = [x_student[src[e]], edge_feat[e], x_item[dst[e]]]   # [320]
    h = elu(x @ W1 + b1)                                    # [256]
    out[e] = h @ W2 + b2 + offset[dst[e]]

Sharding: edges split 8-way (data parallel); node tables + weights
replicated per core. No collectives needed (forward only).

Device strategy per core:
  - Host sorts its edge shard into 16 classes by (src//32768, dst//32768)
    so node-table gathers use int16 indices against a per-class base
    offset (dma_gather transpose-mode custom instruction).
  - 512-index gathers (the SWDGE ucode hard-caps num_idxs at 512; 1024+
    wedges the device) on a SINGLE queue with no chaining. The queue
    worker serializes gathers end-to-end at ~4.5 us each, which is the
    kernel's dominant cost (~2.3 ms of ~2.4 ms total per core).
    Measured-but-rejected alternatives (2026-08-09):
      * multi-queue (nq>=2) with >1 gather in flight: 1.1-1.8 ms BUT
        INTERMITTENTLY CORRUPT (same build passes/fails across runs —
        sem-lane/ring race). Unusable against a correctness gate.
      * nq=4 chained 1-in-flight (old baseline style): safe, 3.8 ms.
      * pmode (prepare_only + per-prep trigger, sem=gdma): pipelines
        desc-gen under DMA drain on one queue. The consumer wait_ge must
        carry explicit dep edges onto the gather-reading matmuls (Tile's
        clock-wait pass otherwise floats the bare wait ~19 matmuls late);
        with that fix (implemented below) it PASSES CoreSim numerics and
        2/2 HW correctness runs (rel 2.4e-3, bit-identical to pmode=False;
        set nc.detect_race_conditions=False — the detector flags benign
        ge-overshoot on the gdma counter). Left OFF: paired same-round HW
        timing measured it ~0.9 ms SLOWER than the default path (median
        diff -909 us, n=12) — per-prep trigger overhead plus the
        block-level gate (all 16 gathers must land before any of the
        block's compute) loses to the default path's per-gather DMASW
        waits, which release each 512-edge window individually. A
        per-window gate (wait 16*(2w+2) before window w) might recover
        it, but is unmeasured.
  - Transpose-mode dma_gather delivers rows feature-on-partition (bf16),
    already in the [K, N] layout the PE needs. Both tables are gathered
    at 256 B/row (x_item no longer widened for the offset).
  - offset[dst] + b2 - sum(W2) is precomputed per-edge on host and
    streamed as two bf16 rows (hi/lo) of the edge-feature block, then
    accumulated into the output psum by a tiny K=2 matmul.
  - b1 rides as a 65th row of the ef weight chunk (rhs row = 1.0), so
    h = x@W1 + b1 lands complete in PSUM and activations need no bias;
    one Exp covers both 128-channel halves of a [128, 1024] psum tile.
  - ELU uses elu(x)+1 = relu(x) + min(exp(x), 1); the min/add pair is a
    single fused scalar_tensor_tensor op (4x DVE mode on bf16). The
    "+1" constant is folded into the streamed offset rows.
  - Per-window [1,512] results are DMA'd from PSUM straight to HBM.
"""
import sys
sys.path.insert(0, "/opt/trn_rl_repo")
from contextlib import ExitStack

import numpy as np
import ml_dtypes

import concourse.bass as bass
from concourse import bacc
import concourse.mybir as mybir
import concourse.tile as tile
from concourse.tile_rust import add_dep_helper
from concourse.bass_utils import run_bass_kernel_spmd

N_NODES = 100000
N_EDGES = 1000000
IN_CH = 128
EDGE_DIM = 64
DEC_CH = 256
N_CORES = 8
E_PER = N_EDGES // N_CORES
BUCKET = 32768
N_BKT = (N_NODES + BUCKET - 1) // BUCKET  # 4
WIN = 512           # psum window (edges per matmul group)
BLK = 4096          # edges per gather instruction / block
EFROWS = EDGE_DIM + 1   # ef rows + ones row (b1 carrier)

BF16 = ml_dtypes.bfloat16


# ---------------------------------------------------------------- host prep

def _class_ids(src, dst):
    return (src // BUCKET) * N_BKT + (dst // BUCKET)


def _prep_cores(src_all, dst_all, ef_all, off_edge_all):
    """Sort each core's edges by (src,dst) bucket class; pad classes to a
    uniform per-class capacity so one SPMD program fits all cores."""
    shards = []
    counts = np.zeros((N_CORES, N_BKT * N_BKT), np.int64)
    for c in range(N_CORES):
        s = slice(c * E_PER, (c + 1) * E_PER)
        src, dst = src_all[s], dst_all[s]
        cls = _class_ids(src, dst)
        order = np.argsort(cls, kind="stable")
        shards.append((src, dst, ef_all[s], off_edge_all[s], cls, order))
        counts[c] = np.bincount(cls, minlength=N_BKT * N_BKT)

    caps = counts.max(axis=0)
    caps = ((caps + WIN - 1) // WIN) * WIN  # pad each class to 512-mult
    e_tot = int(caps.sum())

    blocks = []  # (offset, n, bs, bd, col_base)
    a = 0
    cb = 0
    for k in range(N_BKT * N_BKT):
        cap = int(caps[k])
        while cap > 0:
            n = min(BLK, cap)
            blocks.append((a, n, k // N_BKT, k % N_BKT, cb))
            a += n
            cb += -(-(n // WIN) // 4) * WIN  # ceil(nw/4)*512 output cols
            cap -= n
    cols_tot = cb

    # out/offx live in a [4, cols_tot] layout: window w of a block maps to
    # row w%4, cols [col_base + (w//4)*WIN, +WIN)
    perm = np.empty(e_tot, np.int64)  # sorted-edge position -> flat out idx
    for (a, n, _bs, _bd, cb) in blocks:
        e = np.arange(n)
        w, c = e // WIN, e % WIN
        perm[a:a + n] = (w % 4) * cols_tot + cb + (w // 4) * WIN + c

    per_core = []
    for c in range(N_CORES):
        src, dst, ef, offe, cls, order = shards[c]
        idx_fs = np.zeros(e_tot, np.int16)
        idx_fi = np.zeros(e_tot, np.int16)
        efx = np.zeros((EFROWS, e_tot), np.float32)
        efx[EDGE_DIM] = 1.0  # ones row: carries b1 through the ef matmul
        off_sorted = np.zeros(e_tot, np.float32)
        pos = np.full(e_tot, -1, np.int64)
        a = 0
        cls_sorted = cls[order]
        for k in range(N_BKT * N_BKT):
            sel = order[np.searchsorted(cls_sorted, k):
                        np.searchsorted(cls_sorted, k + 1)]
            nk = len(sel)
            idx_fs[a:a + nk] = (src[sel] - (k // N_BKT) * BUCKET).astype(np.int16)
            idx_fi[a:a + nk] = (dst[sel] - (k % N_BKT) * BUCKET).astype(np.int16)
            efx[:EDGE_DIM, a:a + nk] = ef[sel].T
            off_sorted[a:a + nk] = offe[sel]
            pos[a:a + nk] = sel
            a += int(caps[k])

        def wrap(ii):
            w = ii.reshape(-1, 16).T.copy()          # [16, e_tot/16]
            return np.tile(w, (8, 1))                 # [128, e_tot/16]

        offx = np.zeros(4 * cols_tot, np.float32)
        offx[perm] = off_sorted
        idx2 = np.stack([wrap(idx_fs), wrap(idx_fi)], axis=1)  # [128,2,e_tot/16]
        per_core.append({
            "idx2": np.ascontiguousarray(idx2),
            "efx": efx.astype(BF16),                  # [65, e_tot]
            "offx": offx.reshape(4, cols_tot),
            "pos": pos,
        })
    return blocks, e_tot, cols_tot, perm, per_core


# ---------------------------------------------------------------- device build

_BUILD_CACHE = {}

_LAST_BLOCKS = None


def _build(blocks, e_tot, cols_tot, mode="full", gb=3, hb=3, sb=3,
           scratch=32768, nidx=512, nq=1, repeat=1, chain_d=0, pmode=False):
    key = (tuple(blocks), e_tot, cols_tot, mode, gb, hb, sb, scratch, nidx,
           nq, repeat, chain_d, pmode)
    if key in _BUILD_CACHE:
        return _BUILD_CACHE[key]
    do_gather = mode in ("full", "gather")
    do_compute = mode in ("full", "compute")

    nc = bacc.Bacc("TRN2", num_swdge_queues=nq,
                   dynamic_dma_scratch_size=scratch)
    dt = mybir.dt
    xs_t = nc.dram_tensor("xs", [N_NODES, IN_CH], dt.bfloat16, kind="ExternalInput")
    xi_t = nc.dram_tensor("xi", [N_NODES, IN_CH], dt.bfloat16, kind="ExternalInput")
    idx2_t = nc.dram_tensor("idx2", [128, 2, e_tot // 16], dt.int16, kind="ExternalInput")
    efx = nc.dram_tensor("efx", [EFROWS, e_tot], dt.bfloat16, kind="ExternalInput")
    offx = nc.dram_tensor("offx", [4, cols_tot], dt.float32, kind="ExternalInput")
    # wpack rows: 0:128 fs chunk, 128:193 ef chunk + b1 row, 193:321 fi chunk
    wpack = nc.dram_tensor("wpack", [2 * IN_CH + EDGE_DIM + 1, DEC_CH],
                           dt.bfloat16, kind="ExternalInput")
    w2 = nc.dram_tensor("w2", [DEC_CH], dt.bfloat16, kind="ExternalInput")
    out_d = nc.dram_tensor("out", [4, cols_tot], dt.float32, kind="ExternalOutput")

    with tile.TileContext(nc) as tc, ExitStack() as ctx:
        const = ctx.enter_context(tc.tile_pool(name="const", bufs=1))
        gp = ctx.enter_context(tc.tile_pool(name="gp", bufs=gb))
        ip = ctx.enter_context(tc.tile_pool(name="ip", bufs=2))
        sp = ctx.enter_context(tc.tile_pool(name="sp", bufs=sb))
        hp = ctx.enter_context(tc.tile_pool(name="hp", bufs=hb, space="PSUM"))
        pp = ctx.enter_context(tc.tile_pool(name="pp", bufs=2, space="PSUM"))

        # weights: lhsT blocks [K, M] (K on partitions)
        w1_fs, w1_ef, w1_fi = [], [], []
        for m in range(2):
            ms = slice(m * 128, (m + 1) * 128)
            t = const.tile([128, 128], dt.bfloat16, name=f"w1fs{m}")
            nc.sync.dma_start(t[:], wpack[0:128, ms])
            w1_fs.append(t)
            t = const.tile([65, 128], dt.bfloat16, name=f"w1ef{m}")
            nc.sync.dma_start(t[:], wpack[128:193, ms])
            w1_ef.append(t)
            t = const.tile([128, 128], dt.bfloat16, name=f"w1fi{m}")
            nc.sync.dma_start(t[:], wpack[193:321, ms])
            w1_fi.append(t)
        # w2 as 8 zero-padded [128, 4] lhsT blocks: block (j, m) holds the
        # m-half of w2 in column j. A window with group slot j uses blocks
        # (j, 0) and (j, 1), so its rank-1 result lands in psum row j while
        # rows != j accumulate zeros — four windows share one [4, WIN] bank.
        w2g = const.tile([128, 8, 4], dt.bfloat16)
        nc.vector.memset(w2g[:], 0.0)
        for j in range(4):
            for m in range(2):
                nc.sync.dma_start(w2g[:, j * 2 + m, j:j + 1],
                                  w2[m * 128:(m + 1) * 128])

        gather_state = {"count": 0, "hist": []}
        dma_sem = None
        if pmode:
            # prepare_only gathers: desc-gen runs free of the queue worker's
            # per-instruction drain-wait; each prep is fired by its own
            # trigger and bumps dma_sem by 16 on completion.
            dma_sem = nc.alloc_semaphore("gdma")
            nc.gpsimd.sem_clear(dma_sem)

        def chain(inst):
            """Bound the number of in-flight gathers to chain_d by making
            gather g wait on gather g-chain_d's DMA completion."""
            hist = gather_state["hist"]
            if chain_d and len(hist) >= chain_d:
                add_dep_helper(inst.ins, hist[-chain_d].ins,
                               reason="bound in-flight gathers")
            hist.append(inst)

        def gather(out_ap, src_ap, idx_ap, ns, qn):
            if pmode:
                nc.gpsimd.dma_gather(out_ap, src_ap, idx_ap, ns, ns, IN_CH,
                                     transpose=True, prepare_only=True,
                                     sem=dma_sem, queue_num=qn)
                nc.gpsimd.trigger_dma(count=None, queue_num=qn)
            else:
                chain(nc.gpsimd.dma_gather(out_ap, src_ap, idx_ap, ns, ns,
                                           IN_CH, transpose=True,
                                           queue_num=qn))
            gather_state["count"] += 1

        def emit_loads(blk):
            """DMA this block's streams and issue its gathers."""
            a, n, bs, bd, cb = blk
            nw = n // WIN
            ncols = -(-nw // 4) * WIN
            idx = ip.tile([128, 2, n // 16], dt.int16, tag="idx")
            nc.sync.dma_start(idx[:], idx2_t[:, :, a // 16:(a + n) // 16])
            ifs = idx[:, 0, :]
            ifi = idx[:, 1, :]

            eft = gp.tile([EFROWS, n], dt.bfloat16, tag="ef")
            nc.sync.dma_start(eft[:], efx[:, a:a + n])
            oft = gp.tile([4, ncols], dt.float32, tag="of")
            nc.sync.dma_start(oft[:], offx[:, cb:cb + ncols])
            oacc = gp.tile([4, ncols], dt.float32, tag="oacc")

            fs_g = []
            fi_g = []
            for g0 in range(0, n, BLK):
                ng = min(BLK, n - g0)
                fs_t = gp.tile([128, 1, ng], dt.bfloat16, tag=f"fs{g0}")
                fi_t = gp.tile([128, 1, ng], dt.bfloat16, tag=f"fi{g0}")
                for s0 in range(0, ng, nidx):
                    ns = min(nidx, ng - s0)
                    i0 = g0 + s0
                    if do_gather:
                        g = gather_state["count"]
                        gather(fs_t[:, 0:1, s0:s0 + ns], xs_t[bs * BUCKET:, :],
                               ifs[:, i0 // 16:(i0 + ns) // 16], ns, g % nq)
                        gather(fi_t[:, 0:1, s0:s0 + ns], xi_t[bd * BUCKET:, :],
                               ifi[:, i0 // 16:(i0 + ns) // 16], ns,
                               (g + 1) % nq)
                    else:
                        nc.gpsimd.memset(fs_t[:, 0:1, s0:s0 + ns], 0.5)
                        nc.gpsimd.memset(fi_t[:, 0:1, s0:s0 + ns], 0.5)
                fs_g.append(fs_t)
                fi_g.append(fi_t)
            return (blk, eft, oft, oacc, fs_g, fi_g, gather_state["count"])

        def emit_compute(state):
            (a, n, bs, bd, cb), eft, oft, oacc, fs_g, fi_g, gcnt = state
            nw = n // WIN
            ncols = -(-nw // 4) * WIN
            gate = None
            if pmode and do_gather:
                # gate this block's consumers on its gathers' DMA completion.
                # Tile's clock-wait pass can float a bare wait past consumer
                # matmuls, so every gather-reading matmul gets an explicit
                # dep edge onto the gate.
                gate = nc.tensor.wait_ge(dma_sem, 16 * gcnt)
            o_ps = None
            for w in range(0, n, WIN):
                ws = slice(w, w + WIN)
                gi, gw = w // BLK, w % BLK
                gs = slice(gw, gw + WIN)
                wi = w // WIN
                j = wi % 4
                h = hp.tile([128, 2 * WIN], dt.float32, tag="h", space="PSUM")
                for m in range(2):
                    hm = h[:, m * WIN:(m + 1) * WIN]
                    mm = nc.tensor.matmul(hm, w1_fs[m][:], fs_g[gi][:, 0, gs],
                                          start=True, stop=False)
                    if gate is not None:
                        add_dep_helper(mm.ins, gate.ins, reason="gather gate")
                    nc.tensor.matmul(hm, w1_ef[m][:], eft[0:65, ws],
                                     start=False, stop=False)
                    mm = nc.tensor.matmul(hm, w1_fi[m][:], fi_g[gi][:, 0, gs],
                                          start=False, stop=True)
                    if gate is not None:
                        add_dep_helper(mm.ins, gate.ins, reason="gather gate")
                # elu(x)+1 = relu(x) + min(exp(x), 1); +1 folded into offx
                e_t = sp.tile([128, 2 * WIN], dt.bfloat16, tag="e")
                nc.scalar.activation(e_t[:], h[:], mybir.ActivationFunctionType.Exp)
                r_t = sp.tile([128, 2 * WIN], dt.bfloat16, tag="r")
                nc.scalar.activation(r_t[:, 0:WIN], h[:, 0:WIN],
                                     mybir.ActivationFunctionType.Relu)
                nc.vector.tensor_scalar(out=r_t[:, WIN:2 * WIN],
                                        in0=h[:, WIN:2 * WIN],
                                        scalar1=0.0, scalar2=None,
                                        op0=mybir.AluOpType.max)
                t_t = sp.tile([128, 2 * WIN], dt.bfloat16, tag="t")
                nc.vector.tensor_scalar(out=t_t[:], in0=e_t[:],
                                        scalar1=1.0, scalar2=None,
                                        op0=mybir.AluOpType.min)
                u_t = sp.tile([128, 2 * WIN], dt.bfloat16, tag="u")
                nc.vector.tensor_add(u_t[:], t_t[:], r_t[:])

                if j == 0:
                    o_ps = pp.tile([4, WIN], dt.float32, tag="ops", space="PSUM")
                last = j == 3 or wi == nw - 1
                nc.tensor.matmul(o_ps[:], w2g[:, j * 2, :], u_t[:, 0:WIN],
                                 start=(j == 0), stop=False)
                nc.tensor.matmul(o_ps[:], w2g[:, j * 2 + 1, :], u_t[:, WIN:2 * WIN],
                                 start=False, stop=last)
                if last:
                    # one contiguous [4, WIN] move+add covers up to 4 windows
                    gcol = slice((wi // 4) * WIN, (wi // 4) * WIN + WIN)
                    nc.vector.tensor_add(oacc[0:4, gcol], o_ps[:],
                                         oft[0:4, gcol])

            nc.scalar.dma_start(out_d[:, cb:cb + ncols], oacc[:])

        # software pipeline: block k's gathers issue before block k-1's
        # compute so the in-order Pool engine never parks a gather behind
        # output-move TTs that depend on late compute
        pending = None
        for _rep in range(repeat):
            for blk in blocks:
                state = emit_loads(blk)
                if do_compute and pending is not None:
                    emit_compute(pending)
                pending = state
        if do_compute and pending is not None:
            emit_compute(pending)

    nc.finalize()
    if nq > 1:
        # Align each gather's SWDGE queue with the DMASW sem lane Tile
        # assigned it (lane = scheduled position % 8, lanes are bound to
        # queue lane % 4 in ucode). Post-patching after scheduling beats
        # serializing the gathers to pin the order up front.
        i = 0
        for b in nc.m.functions[0].blocks:
            for ins in b.instructions:
                if type(ins).__name__ == "InstDMAGatherAnt":
                    ins.queue_num = (i % 8) % nq
                    i += 1
    _BUILD_CACHE[key] = nc
    return nc


# ---------------------------------------------------------------- entry points

def prepare(x_student, x_item, edge_label_index, edge_feat, offset, W1, b1, W2, b2):
    """Host prep + program build. Returns (nc, in_maps, metas)."""
    src = np.asarray(edge_label_index[0], np.int64)
    dst = np.asarray(edge_label_index[1], np.int64)
    ef = np.asarray(edge_feat, np.float32)

    w1_bf = np.asarray(W1, np.float32).astype(BF16)
    w2_bf = np.asarray(W2, np.float32).reshape(-1).astype(BF16)
    b1_f = np.asarray(b1, np.float32).reshape(-1)
    b2_f = float(np.asarray(b2, np.float32).reshape(-1)[0])

    # per-edge streamed constant: offset[dst] + b2 - sum(w2)  (the -sum
    # folds the "+1" of elu+1 out of the device-side accumulation)
    off = np.asarray(offset, np.float32).reshape(-1)
    off_edge = off[dst] + (b2_f - float(w2_bf.astype(np.float32).sum()))

    blocks, e_tot, cols_tot, perm, per_core = _prep_cores(src, dst, ef, off_edge)

    xs_bf = np.asarray(x_student, np.float32).astype(BF16)
    xi_bf = np.asarray(x_item, np.float32).astype(BF16)

    wpack = np.zeros((2 * IN_CH + EDGE_DIM + 1, DEC_CH), BF16)
    wpack[0:128] = w1_bf[0:128]
    wpack[128:192] = w1_bf[128:192]
    wpack[192] = b1_f.astype(BF16)          # b1 row (rhs ones row hits it)
    wpack[193:321] = w1_bf[192:320]

    global _LAST_BLOCKS
    _LAST_BLOCKS = (blocks, e_tot, cols_tot)
    nc = _build(tuple(blocks), e_tot, cols_tot)
    in_maps = []
    for c in range(N_CORES):
        pc = per_core[c]
        in_maps.append({
            "xs": xs_bf, "xi": xi_bf,
            "idx2": pc["idx2"],
            "efx": pc["efx"], "offx": pc["offx"],
            "wpack": wpack, "w2": w2_bf,
        })
    metas = [(pc["pos"], perm) for pc in per_core]
    return nc, in_maps, metas


def unshard(results, metas):
    out = np.empty((N_EDGES, 1), np.float32)
    for c in range(N_CORES):
        pos, perm = metas[c]
        valid = pos >= 0
        flat = results[c]["out"].reshape(-1)
        part = np.empty(E_PER, np.float32)
        part[pos[valid]] = flat[perm[valid]]
        out[c * E_PER:(c + 1) * E_PER, 0] = part
    return out


def kernel(x_student, x_item, edge_label_index, edge_feat, offset, W1, b1, W2, b2):
    nc, in_maps, metas = prepare(x_student, x_item, edge_label_index, edge_feat,
                                 offset, W1, b1, W2, b2)
    res = run_bass_kernel_spmd(nc, in_maps, core_ids=list(range(N_CORES)))
    return unshard(res.results, metas)



# revision 4
# speedup vs baseline: 187.7509x; 187.7509x over previous
"""GNN edge-MLP classifier kernel for 8 Trainium2 NeuronCores.

Reference computation (per edge e):
    x = [x_student[src[e]], edge_feat[e], x_item[dst[e]]]   # [320]
    h = elu(x @ W1 + b1)                                    # [256]
    out[e] = h @ W2 + b2 + offset[dst[e]]

Sharding: edges split 8-way (data parallel); node tables + weights
replicated per core. No collectives needed (forward only).

Device strategy per core:
  - Host sorts its edge shard into 16 classes by (src//32768, dst//32768)
    so node-table gathers use int16 indices against a per-class base
    offset (dma_gather transpose-mode custom instruction).
  - 512-index gathers (the SWDGE ucode hard-caps num_idxs at 512; 1024+
    wedges the device) on a SINGLE queue with no chaining. The queue
    worker serializes gathers end-to-end at ~4.5 us each, which is the
    kernel's dominant cost (~2.3 ms of ~2.4 ms total per core).
    Measured-but-rejected alternatives (2026-08-09):
      * multi-queue (nq>=2) with >1 gather in flight: 1.1-1.8 ms BUT
        INTERMITTENTLY CORRUPT (same build passes/fails across runs —
        sem-lane/ring race). Unusable against a correctness gate.
      * nq=4 chained 1-in-flight (old baseline style): safe, 3.8 ms.
      * pmode (prepare_only + per-prep trigger, sem=gdma): pipelines
        desc-gen under DMA drain on one queue. The consumer wait_ge must
        carry explicit dep edges onto the gather-reading matmuls (Tile's
        clock-wait pass otherwise floats the bare wait ~19 matmuls late);
        with that fix (implemented below) it PASSES CoreSim numerics and
        2/2 HW correctness runs (rel 2.4e-3, bit-identical to pmode=False;
        set nc.detect_race_conditions=False — the detector flags benign
        ge-overshoot on the gdma counter). Left OFF: paired same-round HW
        timing measured it ~0.9 ms SLOWER than the default path (median
        diff -909 us, n=12) — per-prep trigger overhead plus the
        block-level gate (all 16 gathers must land before any of the
        block's compute) loses to the default path's per-gather DMASW
        waits, which release each 512-edge window individually. A
        per-window gate (wait 16*(2w+2) before window w) might recover
        it, but is unmeasured.
  - Transpose-mode dma_gather delivers rows feature-on-partition (bf16),
    already in the [K, N] layout the PE needs. Both tables are gathered
    at 256 B/row (x_item no longer widened for the offset).
  - offset[dst] + b2 - sum(W2) is precomputed per-edge on host and
    streamed as two bf16 rows (hi/lo) of the edge-feature block, then
    accumulated into the output psum by a tiny K=2 matmul.
  - b1 rides as a 65th row of the ef weight chunk (rhs row = 1.0), so
    h = x@W1 + b1 lands complete in PSUM and activations need no bias;
    one Exp covers both 128-channel halves of a [128, 1024] psum tile.
  - ELU uses elu(x)+1 = relu(x) + min(exp(x), 1); the min/add pair is a
    single fused scalar_tensor_tensor op (4x DVE mode on bf16). The
    "+1" constant is folded into the streamed offset rows.
  - Per-window [1,512] results are DMA'd from PSUM straight to HBM.
"""
import sys
sys.path.insert(0, "/opt/trn_rl_repo")
from contextlib import ExitStack

import numpy as np
import ml_dtypes

import concourse.bass as bass
from concourse import bacc
import concourse.mybir as mybir
import concourse.tile as tile
from concourse.tile_rust import add_dep_helper
from concourse.bass_utils import run_bass_kernel_spmd

N_NODES = 100000
N_EDGES = 1000000
IN_CH = 128
EDGE_DIM = 64
DEC_CH = 256
N_CORES = 8
E_PER = N_EDGES // N_CORES
BUCKET = 32768
N_BKT = (N_NODES + BUCKET - 1) // BUCKET  # 4
WIN = 512           # psum window (edges per matmul group)
BLK = 4096          # edges per gather instruction / block
EFROWS = EDGE_DIM + 1   # ef rows + ones row (b1 carrier)

BF16 = ml_dtypes.bfloat16


# ---------------------------------------------------------------- host prep

def _class_ids(src, dst):
    return (src // BUCKET) * N_BKT + (dst // BUCKET)


def _prep_cores(src_all, dst_all, ef_all, off_edge_all):
    """Sort each core's edges by (src,dst) bucket class; pad classes to a
    uniform per-class capacity so one SPMD program fits all cores."""
    shards = []
    counts = np.zeros((N_CORES, N_BKT * N_BKT), np.int64)
    for c in range(N_CORES):
        s = slice(c * E_PER, (c + 1) * E_PER)
        src, dst = src_all[s], dst_all[s]
        cls = _class_ids(src, dst)
        order = np.argsort(cls, kind="stable")
        shards.append((src, dst, ef_all[s], off_edge_all[s], cls, order))
        counts[c] = np.bincount(cls, minlength=N_BKT * N_BKT)

    caps = counts.max(axis=0)
    caps = ((caps + WIN - 1) // WIN) * WIN  # pad each class to 512-mult
    e_tot = int(caps.sum())

    blocks = []  # (offset, n, bs, bd, col_base)
    a = 0
    cb = 0
    for k in range(N_BKT * N_BKT):
        cap = int(caps[k])
        while cap > 0:
            n = min(BLK, cap)
            blocks.append((a, n, k // N_BKT, k % N_BKT, cb))
            a += n
            cb += -(-(n // WIN) // 4) * WIN  # ceil(nw/4)*512 output cols
            cap -= n
    cols_tot = cb

    # out/offx live in a [4, cols_tot] layout: window w of a block maps to
    # row w%4, cols [col_base + (w//4)*WIN, +WIN)
    perm = np.empty(e_tot, np.int64)  # sorted-edge position -> flat out idx
    for (a, n, _bs, _bd, cb) in blocks:
        e = np.arange(n)
        w, c = e // WIN, e % WIN
        perm[a:a + n] = (w % 4) * cols_tot + cb + (w // 4) * WIN + c

    per_core = []
    for c in range(N_CORES):
        src, dst, ef, offe, cls, order = shards[c]
        idx_fs = np.zeros(e_tot, np.int16)
        idx_fi = np.zeros(e_tot, np.int16)
        efx = np.zeros((EFROWS, e_tot), np.float32)
        efx[EDGE_DIM] = 1.0  # ones row: carries b1 through the ef matmul
        off_sorted = np.zeros(e_tot, np.float32)
        pos = np.full(e_tot, -1, np.int64)
        a = 0
        cls_sorted = cls[order]
        for k in range(N_BKT * N_BKT):
            sel = order[np.searchsorted(cls_sorted, k):
                        np.searchsorted(cls_sorted, k + 1)]
            nk = len(sel)
            idx_fs[a:a + nk] = (src[sel] - (k // N_BKT) * BUCKET).astype(np.int16)
            idx_fi[a:a + nk] = (dst[sel] - (k % N_BKT) * BUCKET).astype(np.int16)
            efx[:EDGE_DIM, a:a + nk] = ef[sel].T
            off_sorted[a:a + nk] = offe[sel]
            pos[a:a + nk] = sel
            a += int(caps[k])

        def wrap(ii):
            w = ii.reshape(-1, 16).T.copy()          # [16, e_tot/16]
            return np.tile(w, (8, 1))                 # [128, e_tot/16]

        offx = np.zeros(4 * cols_tot, np.float32)
        offx[perm] = off_sorted
        idx2 = np.stack([wrap(idx_fs), wrap(idx_fi)], axis=1)  # [128,2,e_tot/16]
        per_core.append({
            "idx2": np.ascontiguousarray(idx2),
            "efx": efx.astype(BF16),                  # [65, e_tot]
            "offx": offx.reshape(4, cols_tot),
            "pos": pos,
        })
    return blocks, e_tot, cols_tot, perm, per_core


# ---------------------------------------------------------------- device build

_BUILD_CACHE = {}

_LAST_BLOCKS = None


def _build(blocks, e_tot, cols_tot, mode="full", gb=3, hb=3, sb=3,
           scratch=32768, nidx=256, nq=1, repeat=1, chain_d=0, pmode=False):
    key = (tuple(blocks), e_tot, cols_tot, mode, gb, hb, sb, scratch, nidx,
           nq, repeat, chain_d, pmode)
    if key in _BUILD_CACHE:
        return _BUILD_CACHE[key]
    do_gather = mode in ("full", "gather")
    do_compute = mode in ("full", "compute")

    nc = bacc.Bacc("TRN2", num_swdge_queues=nq,
                   dynamic_dma_scratch_size=scratch)
    dt = mybir.dt
    xs_t = nc.dram_tensor("xs", [N_NODES, IN_CH], dt.bfloat16, kind="ExternalInput")
    xi_t = nc.dram_tensor("xi", [N_NODES, IN_CH], dt.bfloat16, kind="ExternalInput")
    idx2_t = nc.dram_tensor("idx2", [128, 2, e_tot // 16], dt.int16, kind="ExternalInput")
    efx = nc.dram_tensor("efx", [EFROWS, e_tot], dt.bfloat16, kind="ExternalInput")
    offx = nc.dram_tensor("offx", [4, cols_tot], dt.float32, kind="ExternalInput")
    # wpack rows: 0:128 fs chunk, 128:193 ef chunk + b1 row, 193:321 fi chunk
    wpack = nc.dram_tensor("wpack", [2 * IN_CH + EDGE_DIM + 1, DEC_CH],
                           dt.bfloat16, kind="ExternalInput")
    w2 = nc.dram_tensor("w2", [DEC_CH], dt.bfloat16, kind="ExternalInput")
    out_d = nc.dram_tensor("out", [4, cols_tot], dt.float32, kind="ExternalOutput")

    with tile.TileContext(nc) as tc, ExitStack() as ctx:
        const = ctx.enter_context(tc.tile_pool(name="const", bufs=1))
        gp = ctx.enter_context(tc.tile_pool(name="gp", bufs=gb))
        ip = ctx.enter_context(tc.tile_pool(name="ip", bufs=2))
        sp = ctx.enter_context(tc.tile_pool(name="sp", bufs=sb))
        hp = ctx.enter_context(tc.tile_pool(name="hp", bufs=hb, space="PSUM"))
        pp = ctx.enter_context(tc.tile_pool(name="pp", bufs=2, space="PSUM"))

        # weights: lhsT blocks [K, M] (K on partitions)
        w1_fs, w1_ef, w1_fi = [], [], []
        for m in range(2):
            ms = slice(m * 128, (m + 1) * 128)
            t = const.tile([128, 128], dt.bfloat16, name=f"w1fs{m}")
            nc.sync.dma_start(t[:], wpack[0:128, ms])
            w1_fs.append(t)
            t = const.tile([65, 128], dt.bfloat16, name=f"w1ef{m}")
            nc.sync.dma_start(t[:], wpack[128:193, ms])
            w1_ef.append(t)
            t = const.tile([128, 128], dt.bfloat16, name=f"w1fi{m}")
            nc.sync.dma_start(t[:], wpack[193:321, ms])
            w1_fi.append(t)
        # w2 as 8 zero-padded [128, 4] lhsT blocks: block (j, m) holds the
        # m-half of w2 in column j. A window with group slot j uses blocks
        # (j, 0) and (j, 1), so its rank-1 result lands in psum row j while
        # rows != j accumulate zeros — four windows share one [4, WIN] bank.
        w2g = const.tile([128, 8, 4], dt.bfloat16)
        nc.vector.memset(w2g[:], 0.0)
        for j in range(4):
            for m in range(2):
                nc.sync.dma_start(w2g[:, j * 2 + m, j:j + 1],
                                  w2[m * 128:(m + 1) * 128])

        gather_state = {"count": 0, "hist": []}
        dma_sem = None
        if pmode:
            # prepare_only gathers: desc-gen runs free of the queue worker's
            # per-instruction drain-wait; each prep is fired by its own
            # trigger and bumps dma_sem by 16 on completion.
            dma_sem = nc.alloc_semaphore("gdma")
            nc.gpsimd.sem_clear(dma_sem)

        def chain(inst):
            """Bound the number of in-flight gathers to chain_d by making
            gather g wait on gather g-chain_d's DMA completion."""
            hist = gather_state["hist"]
            if chain_d and len(hist) >= chain_d:
                add_dep_helper(inst.ins, hist[-chain_d].ins,
                               reason="bound in-flight gathers")
            hist.append(inst)

        def gather(out_ap, src_ap, idx_ap, ns, qn):
            if pmode:
                nc.gpsimd.dma_gather(out_ap, src_ap, idx_ap, ns, ns, IN_CH,
                                     transpose=True, prepare_only=True,
                                     sem=dma_sem, queue_num=qn)
                nc.gpsimd.trigger_dma(count=None, queue_num=qn)
            else:
                chain(nc.gpsimd.dma_gather(out_ap, src_ap, idx_ap, ns, ns,
                                           IN_CH, transpose=True,
                                           queue_num=qn))
            gather_state["count"] += 1

        def emit_loads(blk):
            """DMA this block's streams and issue its gathers."""
            a, n, bs, bd, cb = blk
            nw = n // WIN
            ncols = -(-nw // 4) * WIN
            idx = ip.tile([128, 2, n // 16], dt.int16, tag="idx")
            nc.sync.dma_start(idx[:], idx2_t[:, :, a // 16:(a + n) // 16])
            ifs = idx[:, 0, :]
            ifi = idx[:, 1, :]

            eft = gp.tile([EFROWS, n], dt.bfloat16, tag="ef")
            nc.sync.dma_start(eft[:], efx[:, a:a + n])
            oft = gp.tile([4, ncols], dt.float32, tag="of")
            nc.sync.dma_start(oft[:], offx[:, cb:cb + ncols])
            oacc = gp.tile([4, ncols], dt.float32, tag="oacc")

            fs_g = []
            fi_g = []
            for g0 in range(0, n, BLK):
                ng = min(BLK, n - g0)
                fs_t = gp.tile([128, 1, ng], dt.bfloat16, tag=f"fs{g0}")
                fi_t = gp.tile([128, 1, ng], dt.bfloat16, tag=f"fi{g0}")
                for s0 in range(0, ng, nidx):
                    ns = min(nidx, ng - s0)
                    i0 = g0 + s0
                    if do_gather:
                        g = gather_state["count"]
                        gather(fs_t[:, 0:1, s0:s0 + ns], xs_t[bs * BUCKET:, :],
                               ifs[:, i0 // 16:(i0 + ns) // 16], ns, g % nq)
                        gather(fi_t[:, 0:1, s0:s0 + ns], xi_t[bd * BUCKET:, :],
                               ifi[:, i0 // 16:(i0 + ns) // 16], ns,
                               (g + 1) % nq)
                    else:
                        nc.gpsimd.memset(fs_t[:, 0:1, s0:s0 + ns], 0.5)
                        nc.gpsimd.memset(fi_t[:, 0:1, s0:s0 + ns], 0.5)
                fs_g.append(fs_t)
                fi_g.append(fi_t)
            return (blk, eft, oft, oacc, fs_g, fi_g, gather_state["count"])

        def emit_compute(state):
            (a, n, bs, bd, cb), eft, oft, oacc, fs_g, fi_g, gcnt = state
            nw = n // WIN
            ncols = -(-nw // 4) * WIN
            gate = None
            if pmode and do_gather:
                # gate this block's consumers on its gathers' DMA completion.
                # Tile's clock-wait pass can float a bare wait past consumer
                # matmuls, so every gather-reading matmul gets an explicit
                # dep edge onto the gate.
                gate = nc.tensor.wait_ge(dma_sem, 16 * gcnt)
            o_ps = None
            for w in range(0, n, WIN):
                ws = slice(w, w + WIN)
                gi, gw = w // BLK, w % BLK
                gs = slice(gw, gw + WIN)
                wi = w // WIN
                j = wi % 4
                h = hp.tile([128, 2 * WIN], dt.float32, tag="h", space="PSUM")
                for m in range(2):
                    hm = h[:, m * WIN:(m + 1) * WIN]
                    mm = nc.tensor.matmul(hm, w1_fs[m][:], fs_g[gi][:, 0, gs],
                                          start=True, stop=False)
                    if gate is not None:
                        add_dep_helper(mm.ins, gate.ins, reason="gather gate")
                    nc.tensor.matmul(hm, w1_ef[m][:], eft[0:65, ws],
                                     start=False, stop=False)
                    mm = nc.tensor.matmul(hm, w1_fi[m][:], fi_g[gi][:, 0, gs],
                                          start=False, stop=True)
                    if gate is not None:
                        add_dep_helper(mm.ins, gate.ins, reason="gather gate")
                # elu(x)+1 = relu(x) + min(exp(x), 1); +1 folded into offx
                e_t = sp.tile([128, 2 * WIN], dt.bfloat16, tag="e")
                nc.scalar.activation(e_t[:], h[:], mybir.ActivationFunctionType.Exp)
                r_t = sp.tile([128, 2 * WIN], dt.bfloat16, tag="r")
                nc.scalar.activation(r_t[:, 0:WIN], h[:, 0:WIN],
                                     mybir.ActivationFunctionType.Relu)
                nc.vector.tensor_scalar(out=r_t[:, WIN:2 * WIN],
                                        in0=h[:, WIN:2 * WIN],
                                        scalar1=0.0, scalar2=None,
                                        op0=mybir.AluOpType.max)
                t_t = sp.tile([128, 2 * WIN], dt.bfloat16, tag="t")
                nc.vector.tensor_scalar(out=t_t[:], in0=e_t[:],
                                        scalar1=1.0, scalar2=None,
                                        op0=mybir.AluOpType.min)
                u_t = sp.tile([128, 2 * WIN], dt.bfloat16, tag="u")
                nc.vector.tensor_add(u_t[:], t_t[:], r_t[:])

                if j == 0:
                    o_ps = pp.tile([4, WIN], dt.float32, tag="ops", space="PSUM")
                last = j == 3 or wi == nw - 1
                nc.tensor.matmul(o_ps[:], w2g[:, j * 2, :], u_t[:, 0:WIN],
                                 start=(j == 0), stop=False)
                nc.tensor.matmul(o_ps[:], w2g[:, j * 2 + 1, :], u_t[:, WIN:2 * WIN],
                                 start=False, stop=last)
                if last:
                    # one contiguous [4, WIN] move+add covers up to 4 windows
                    gcol = slice((wi // 4) * WIN, (wi // 4) * WIN + WIN)
                    nc.vector.tensor_add(oacc[0:4, gcol], o_ps[:],
                                         oft[0:4, gcol])

            nc.scalar.dma_start(out_d[:, cb:cb + ncols], oacc[:])

        # software pipeline: block k's gathers issue before block k-1's
        # compute so the in-order Pool engine never parks a gather behind
        # output-move TTs that depend on late compute
        pending = None
        for _rep in range(repeat):
            for blk in blocks:
                state = emit_loads(blk)
                if do_compute and pending is not None:
                    emit_compute(pending)
                pending = state
        if do_compute and pending is not None:
            emit_compute(pending)

    nc.finalize()
    if nq > 1 and False:
        # Align each gather's SWDGE queue with the DMASW sem lane Tile
        # assigned it (lane = scheduled position % 8, lanes are bound to
        # queue lane % 4 in ucode). Post-patching after scheduling beats
        # serializing the gathers to pin the order up front.
        i = 0
        for b in nc.m.functions[0].blocks:
            for ins in b.instructions:
                if type(ins).__name__ == "InstDMAGatherAnt":
                    ins.queue_num = (i % 8) % nq
                    i += 1
    _BUILD_CACHE[key] = nc
    return nc


# ---------------------------------------------------------------- entry points

def prepare(x_student, x_item, edge_label_index, edge_feat, offset, W1, b1, W2, b2):
    """Host prep + program build. Returns (nc, in_maps, metas)."""
    src = np.asarray(edge_label_index[0], np.int64)
    dst = np.asarray(edge_label_index[1], np.int64)
    ef = np.asarray(edge_feat, np.float32)

    w1_bf = np.asarray(W1, np.float32).astype(BF16)
    w2_bf = np.asarray(W2, np.float32).reshape(-1).astype(BF16)
    b1_f = np.asarray(b1, np.float32).reshape(-1)
    b2_f = float(np.asarray(b2, np.float32).reshape(-1)[0])

    # per-edge streamed constant: offset[dst] + b2 - sum(w2)  (the -sum
    # folds the "+1" of elu+1 out of the device-side accumulation)
    off = np.asarray(offset, np.float32).reshape(-1)
    off_edge = off[dst] + (b2_f - float(w2_bf.astype(np.float32).sum()))

    blocks, e_tot, cols_tot, perm, per_core = _prep_cores(src, dst, ef, off_edge)

    xs_bf = np.asarray(x_student, np.float32).astype(BF16)
    xi_bf = np.asarray(x_item, np.float32).astype(BF16)

    wpack = np.zeros((2 * IN_CH + EDGE_DIM + 1, DEC_CH), BF16)
    wpack[0:128] = w1_bf[0:128]
    wpack[128:192] = w1_bf[128:192]
    wpack[192] = b1_f.astype(BF16)          # b1 row (rhs ones row hits it)
    wpack[193:321] = w1_bf[192:320]

    global _LAST_BLOCKS
    _LAST_BLOCKS = (blocks, e_tot, cols_tot)
    nc = _build(tuple(blocks), e_tot, cols_tot)
    in_maps = []
    for c in range(N_CORES):
        pc = per_core[c]
        in_maps.append({
            "xs": xs_bf, "xi": xi_bf,
            "idx2": pc["idx2"],
            "efx": pc["efx"], "offx": pc["offx"],
            "wpack": wpack, "w2": w2_bf,
        })
    metas = [(pc["pos"], perm) for pc in per_core]
    return nc, in_maps, metas


def unshard(results, metas):
    out = np.empty((N_EDGES, 1), np.float32)
    for c in range(N_CORES):
        pos, perm = metas[c]
        valid = pos >= 0
        flat = results[c]["out"].reshape(-1)
        part = np.empty(E_PER, np.float32)
        part[pos[valid]] = flat[perm[valid]]
        out[c * E_PER:(c + 1) * E_PER, 0] = part
    return out


def kernel(x_student, x_item, edge_label_index, edge_feat, offset, W1, b1, W2, b2):
    nc, in_maps, metas = prepare(x_student, x_item, edge_label_index, edge_feat,
                                 offset, W1, b1, W2, b2)
    res = run_bass_kernel_spmd(nc, in_maps, core_ids=list(range(N_CORES)))
    return unshard(res.results, metas)



# revision 5
# speedup vs baseline: 225.3624x; 1.2003x over previous
"""GNN edge-MLP classifier kernel for 8 Trainium2 NeuronCores.

Reference computation (per edge e):
    x = [x_student[src[e]], edge_feat[e], x_item[dst[e]]]   # [320]
    h = elu(x @ W1 + b1)                                    # [256]
    out[e] = h @ W2 + b2 + offset[dst[e]]

Sharding: edges split 8-way (data parallel); node tables + weights
replicated per core. No collectives needed (forward only).

Device strategy per core:
  - Host sorts its edge shard into 16 classes by (src//32768, dst//32768)
    so node-table gathers use int16 indices against a per-class base
    offset (dma_gather transpose-mode custom instruction).
  - 512-index gathers (the SWDGE ucode hard-caps num_idxs at 512; 1024+
    wedges the device) on a SINGLE queue with no chaining. The queue
    worker serializes gathers end-to-end at ~4.5 us each, which is the
    kernel's dominant cost (~2.3 ms of ~2.4 ms total per core).
    Measured-but-rejected alternatives (2026-08-09):
      * multi-queue (nq>=2) with >1 gather in flight: 1.1-1.8 ms BUT
        INTERMITTENTLY CORRUPT (same build passes/fails across runs —
        sem-lane/ring race). Unusable against a correctness gate.
      * nq=4 chained 1-in-flight (old baseline style): safe, 3.8 ms.
      * pmode (prepare_only + per-prep trigger, sem=gdma): pipelines
        desc-gen under DMA drain on one queue. The consumer wait_ge must
        carry explicit dep edges onto the gather-reading matmuls (Tile's
        clock-wait pass otherwise floats the bare wait ~19 matmuls late);
        with that fix (implemented below) it PASSES CoreSim numerics and
        2/2 HW correctness runs (rel 2.4e-3, bit-identical to pmode=False;
        set nc.detect_race_conditions=False — the detector flags benign
        ge-overshoot on the gdma counter). Left OFF: paired same-round HW
        timing measured it ~0.9 ms SLOWER than the default path (median
        diff -909 us, n=12) — per-prep trigger overhead plus the
        block-level gate (all 16 gathers must land before any of the
        block's compute) loses to the default path's per-gather DMASW
        waits, which release each 512-edge window individually. A
        per-window gate (wait 16*(2w+2) before window w) might recover
        it, but is unmeasured.
  - Transpose-mode dma_gather delivers rows feature-on-partition (bf16),
    already in the [K, N] layout the PE needs. Both tables are gathered
    at 256 B/row (x_item no longer widened for the offset).
  - offset[dst] + b2 - sum(W2) is precomputed per-edge on host and
    streamed as two bf16 rows (hi/lo) of the edge-feature block, then
    accumulated into the output psum by a tiny K=2 matmul.
  - b1 rides as a 65th row of the ef weight chunk (rhs row = 1.0), so
    h = x@W1 + b1 lands complete in PSUM and activations need no bias;
    one Exp covers both 128-channel halves of a [128, 1024] psum tile.
  - ELU uses elu(x)+1 = relu(x) + min(exp(x), 1); the min/add pair is a
    single fused scalar_tensor_tensor op (4x DVE mode on bf16). The
    "+1" constant is folded into the streamed offset rows.
  - Per-window [1,512] results are DMA'd from PSUM straight to HBM.
"""
import sys
sys.path.insert(0, "/opt/trn_rl_repo")
from contextlib import ExitStack

import numpy as np
import ml_dtypes

import concourse.bass as bass
from concourse import bacc
import concourse.mybir as mybir
import concourse.tile as tile
from concourse.tile_rust import add_dep_helper
from concourse.bass_utils import run_bass_kernel_spmd

N_NODES = 100000
N_EDGES = 1000000
IN_CH = 128
EDGE_DIM = 64
DEC_CH = 256
N_CORES = 8
E_PER = N_EDGES // N_CORES
BUCKET = 32768
N_BKT = (N_NODES + BUCKET - 1) // BUCKET  # 4
WIN = 512           # psum window (edges per matmul group)
BLK = 4096          # edges per gather instruction / block
EFROWS = EDGE_DIM + 1   # ef rows + ones row (b1 carrier)

BF16 = ml_dtypes.bfloat16


# ---------------------------------------------------------------- host prep

def _class_ids(src, dst):
    return (src // BUCKET) * N_BKT + (dst // BUCKET)


def _prep_cores(src_all, dst_all, ef_all, off_edge_all):
    """Sort each core's edges by (src,dst) bucket class; pad classes to a
    uniform per-class capacity so one SPMD program fits all cores."""
    shards = []
    counts = np.zeros((N_CORES, N_BKT * N_BKT), np.int64)
    for c in range(N_CORES):
        s = slice(c * E_PER, (c + 1) * E_PER)
        src, dst = src_all[s], dst_all[s]
        cls = _class_ids(src, dst)
        order = np.argsort(cls, kind="stable")
        shards.append((src, dst, ef_all[s], off_edge_all[s], cls, order))
        counts[c] = np.bincount(cls, minlength=N_BKT * N_BKT)

    caps = counts.max(axis=0)
    caps = ((caps + WIN - 1) // WIN) * WIN  # pad each class to 512-mult
    e_tot = int(caps.sum())

    blocks = []  # (offset, n, bs, bd, col_base)
    a = 0
    cb = 0
    for k in range(N_BKT * N_BKT):
        cap = int(caps[k])
        while cap > 0:
            n = min(BLK, cap)
            blocks.append((a, n, k // N_BKT, k % N_BKT, cb))
            a += n
            cb += -(-(n // WIN) // 4) * WIN  # ceil(nw/4)*512 output cols
            cap -= n
    cols_tot = cb

    # out/offx live in a [4, cols_tot] layout: window w of a block maps to
    # row w%4, cols [col_base + (w//4)*WIN, +WIN)
    perm = np.empty(e_tot, np.int64)  # sorted-edge position -> flat out idx
    for (a, n, _bs, _bd, cb) in blocks:
        e = np.arange(n)
        w, c = e // WIN, e % WIN
        perm[a:a + n] = (w % 4) * cols_tot + cb + (w // 4) * WIN + c

    per_core = []
    for c in range(N_CORES):
        src, dst, ef, offe, cls, order = shards[c]
        idx_fs = np.zeros(e_tot, np.int16)
        idx_fi = np.zeros(e_tot, np.int16)
        efx = np.zeros((EFROWS, e_tot), np.float32)
        efx[EDGE_DIM] = 1.0  # ones row: carries b1 through the ef matmul
        off_sorted = np.zeros(e_tot, np.float32)
        pos = np.full(e_tot, -1, np.int64)
        a = 0
        cls_sorted = cls[order]
        for k in range(N_BKT * N_BKT):
            sel = order[np.searchsorted(cls_sorted, k):
                        np.searchsorted(cls_sorted, k + 1)]
            nk = len(sel)
            idx_fs[a:a + nk] = (src[sel] - (k // N_BKT) * BUCKET).astype(np.int16)
            idx_fi[a:a + nk] = (dst[sel] - (k % N_BKT) * BUCKET).astype(np.int16)
            efx[:EDGE_DIM, a:a + nk] = ef[sel].T
            off_sorted[a:a + nk] = offe[sel]
            pos[a:a + nk] = sel
            a += int(caps[k])

        def wrap(ii):
            w = ii.reshape(-1, 16).T.copy()          # [16, e_tot/16]
            return np.tile(w, (8, 1))                 # [128, e_tot/16]

        offx = np.zeros(4 * cols_tot, np.float32)
        offx[perm] = off_sorted
        idx2 = np.stack([wrap(idx_fs), wrap(idx_fi)], axis=1)  # [128,2,e_tot/16]
        per_core.append({
            "idx2": np.ascontiguousarray(idx2),
            "efx": efx.astype(BF16),                  # [65, e_tot]
            "offx": offx.reshape(4, cols_tot),
            "pos": pos,
        })
    return blocks, e_tot, cols_tot, perm, per_core


# ---------------------------------------------------------------- device build

_BUILD_CACHE = {}

_LAST_BLOCKS = None


def _build(blocks, e_tot, cols_tot, mode="full", gb=3, hb=3, sb=3,
           scratch=32768, nidx=512, nq=1, repeat=1, chain_d=0, pmode=False):
    key = (tuple(blocks), e_tot, cols_tot, mode, gb, hb, sb, scratch, nidx,
           nq, repeat, chain_d, pmode)
    if key in _BUILD_CACHE:
        return _BUILD_CACHE[key]
    do_gather = mode in ("full", "gather")
    do_compute = mode in ("full", "compute")

    nc = bacc.Bacc("TRN2", num_swdge_queues=nq,
                   dynamic_dma_scratch_size=scratch)
    dt = mybir.dt
    xs_t = nc.dram_tensor("xs", [N_NODES, IN_CH], dt.bfloat16, kind="ExternalInput")
    xi_t = nc.dram_tensor("xi", [N_NODES, IN_CH], dt.bfloat16, kind="ExternalInput")
    idx2_t = nc.dram_tensor("idx2", [128, 2, e_tot // 16], dt.int16, kind="ExternalInput")
    efx = nc.dram_tensor("efx", [EFROWS, e_tot], dt.bfloat16, kind="ExternalInput")
    offx = nc.dram_tensor("offx", [4, cols_tot], dt.float32, kind="ExternalInput")
    # wpack rows: 0:128 fs chunk, 128:193 ef chunk + b1 row, 193:321 fi chunk
    wpack = nc.dram_tensor("wpack", [2 * IN_CH + EDGE_DIM + 1, DEC_CH],
                           dt.bfloat16, kind="ExternalInput")
    w2 = nc.dram_tensor("w2", [DEC_CH], dt.bfloat16, kind="ExternalInput")
    out_d = nc.dram_tensor("out", [4, cols_tot], dt.float32, kind="ExternalOutput")

    with tile.TileContext(nc) as tc, ExitStack() as ctx:
        const = ctx.enter_context(tc.tile_pool(name="const", bufs=1))
        gp = ctx.enter_context(tc.tile_pool(name="gp", bufs=gb))
        ip = ctx.enter_context(tc.tile_pool(name="ip", bufs=2))
        sp = ctx.enter_context(tc.tile_pool(name="sp", bufs=sb))
        hp = ctx.enter_context(tc.tile_pool(name="hp", bufs=hb, space="PSUM"))
        pp = ctx.enter_context(tc.tile_pool(name="pp", bufs=2, space="PSUM"))

        # weights: lhsT blocks [K, M] (K on partitions)
        w1_fs, w1_ef, w1_fi = [], [], []
        for m in range(2):
            ms = slice(m * 128, (m + 1) * 128)
            t = const.tile([128, 128], dt.bfloat16, name=f"w1fs{m}")
            nc.sync.dma_start(t[:], wpack[0:128, ms])
            w1_fs.append(t)
            t = const.tile([65, 128], dt.bfloat16, name=f"w1ef{m}")
            nc.sync.dma_start(t[:], wpack[128:193, ms])
            w1_ef.append(t)
            t = const.tile([128, 128], dt.bfloat16, name=f"w1fi{m}")
            nc.sync.dma_start(t[:], wpack[193:321, ms])
            w1_fi.append(t)
        # w2 as 8 zero-padded [128, 4] lhsT blocks: block (j, m) holds the
        # m-half of w2 in column j. A window with group slot j uses blocks
        # (j, 0) and (j, 1), so its rank-1 result lands in psum row j while
        # rows != j accumulate zeros — four windows share one [4, WIN] bank.
        w2g = const.tile([128, 8, 4], dt.bfloat16)
        nc.vector.memset(w2g[:], 0.0)
        for j in range(4):
            for m in range(2):
                nc.sync.dma_start(w2g[:, j * 2 + m, j:j + 1],
                                  w2[m * 128:(m + 1) * 128])

        gather_state = {"count": 0, "hist": []}
        dma_sem = None
        if pmode:
            # prepare_only gathers: desc-gen runs free of the queue worker's
            # per-instruction drain-wait; each prep is fired by its own
            # trigger and bumps dma_sem by 16 on completion.
            dma_sem = nc.alloc_semaphore("gdma")
            nc.gpsimd.sem_clear(dma_sem)

        def chain(inst):
            """Bound the number of in-flight gathers to chain_d by making
            gather g wait on gather g-chain_d's DMA completion."""
            hist = gather_state["hist"]
            if chain_d and len(hist) >= chain_d:
                add_dep_helper(inst.ins, hist[-chain_d].ins,
                               reason="bound in-flight gathers")
            hist.append(inst)

        def gather(out_ap, src_ap, idx_ap, ns, qn):
            if pmode:
                nc.gpsimd.dma_gather(out_ap, src_ap, idx_ap, ns, ns, IN_CH,
                                     transpose=True, prepare_only=True,
                                     sem=dma_sem, queue_num=qn)
                nc.gpsimd.trigger_dma(count=None, queue_num=qn)
            else:
                chain(nc.gpsimd.dma_gather(out_ap, src_ap, idx_ap, ns, ns,
                                           IN_CH, transpose=True,
                                           queue_num=qn))
            gather_state["count"] += 1

        def emit_loads(blk):
            """DMA this block's streams and issue its gathers."""
            a, n, bs, bd, cb = blk
            nw = n // WIN
            ncols = -(-nw // 4) * WIN
            idx = ip.tile([128, 2, n // 16], dt.int16, tag="idx")
            nc.sync.dma_start(idx[:], idx2_t[:, :, a // 16:(a + n) // 16])
            ifs = idx[:, 0, :]
            ifi = idx[:, 1, :]

            eft = gp.tile([EFROWS, n], dt.bfloat16, tag="ef")
            nc.sync.dma_start(eft[:], efx[:, a:a + n])
            oft = gp.tile([4, ncols], dt.float32, tag="of")
            nc.sync.dma_start(oft[:], offx[:, cb:cb + ncols])
            oacc = gp.tile([4, ncols], dt.float32, tag="oacc")

            fs_g = []
            fi_g = []
            for g0 in range(0, n, BLK):
                ng = min(BLK, n - g0)
                fs_t = gp.tile([128, 1, ng], dt.bfloat16, tag=f"fs{g0}")
                fi_t = gp.tile([128, 1, ng], dt.bfloat16, tag=f"fi{g0}")
                for s0 in range(0, ng, nidx):
                    ns = min(nidx, ng - s0)
                    i0 = g0 + s0
                    if do_gather:
                        g = gather_state["count"]
                        gather(fs_t[:, 0:1, s0:s0 + ns], xs_t[bs * BUCKET:, :],
                               ifs[:, i0 // 16:(i0 + ns) // 16], ns, g % nq)
                        gather(fi_t[:, 0:1, s0:s0 + ns], xi_t[bd * BUCKET:, :],
                               ifi[:, i0 // 16:(i0 + ns) // 16], ns,
                               (g + 1) % nq)
                    else:
                        nc.gpsimd.memset(fs_t[:, 0:1, s0:s0 + ns], 0.5)
                        nc.gpsimd.memset(fi_t[:, 0:1, s0:s0 + ns], 0.5)
                fs_g.append(fs_t)
                fi_g.append(fi_t)
            return (blk, eft, oft, oacc, fs_g, fi_g, gather_state["count"])

        def emit_compute(state):
            (a, n, bs, bd, cb), eft, oft, oacc, fs_g, fi_g, gcnt = state
            nw = n // WIN
            ncols = -(-nw // 4) * WIN
            gate = None
            if pmode and do_gather:
                # gate this block's consumers on its gathers' DMA completion.
                # Tile's clock-wait pass can float a bare wait past consumer
                # matmuls, so every gather-reading matmul gets an explicit
                # dep edge onto the gate.
                gate = nc.tensor.wait_ge(dma_sem, 16 * gcnt)
            o_ps = None
            for w in range(0, n, WIN):
                ws = slice(w, w + WIN)
                gi, gw = w // BLK, w % BLK
                gs = slice(gw, gw + WIN)
                wi = w // WIN
                j = wi % 4
                h = hp.tile([128, 2 * WIN], dt.float32, tag="h", space="PSUM")
                for m in range(2):
                    hm = h[:, m * WIN:(m + 1) * WIN]
                    mm = nc.tensor.matmul(hm, w1_fs[m][:], fs_g[gi][:, 0, gs],
                                          start=True, stop=False)
                    if gate is not None:
                        add_dep_helper(mm.ins, gate.ins, reason="gather gate")
                    nc.tensor.matmul(hm, w1_ef[m][:], eft[0:65, ws],
                                     start=False, stop=False)
                    mm = nc.tensor.matmul(hm, w1_fi[m][:], fi_g[gi][:, 0, gs],
                                          start=False, stop=True)
                    if gate is not None:
                        add_dep_helper(mm.ins, gate.ins, reason="gather gate")
                # elu(x)+1 = relu(x) + min(exp(x), 1); +1 folded into offx
                e_t = sp.tile([128, 2 * WIN], dt.bfloat16, tag="e")
                nc.scalar.activation(e_t[:], h[:], mybir.ActivationFunctionType.Exp)
                r_t = sp.tile([128, 2 * WIN], dt.bfloat16, tag="r")
                nc.scalar.activation(r_t[:, 0:WIN], h[:, 0:WIN],
                                     mybir.ActivationFunctionType.Relu)
                nc.vector.tensor_scalar(out=r_t[:, WIN:2 * WIN],
                                        in0=h[:, WIN:2 * WIN],
                                        scalar1=0.0, scalar2=None,
                                        op0=mybir.AluOpType.max)
                t_t = sp.tile([128, 2 * WIN], dt.bfloat16, tag="t")
                nc.vector.tensor_scalar(out=t_t[:], in0=e_t[:],
                                        scalar1=1.0, scalar2=None,
                                        op0=mybir.AluOpType.min)
                u_t = sp.tile([128, 2 * WIN], dt.bfloat16, tag="u")
                nc.vector.tensor_add(u_t[:], t_t[:], r_t[:])

                if j == 0:
                    o_ps = pp.tile([4, WIN], dt.float32, tag="ops", space="PSUM")
                last = j == 3 or wi == nw - 1
                nc.tensor.matmul(o_ps[:], w2g[:, j * 2, :], u_t[:, 0:WIN],
                                 start=(j == 0), stop=False)
                nc.tensor.matmul(o_ps[:], w2g[:, j * 2 + 1, :], u_t[:, WIN:2 * WIN],
                                 start=False, stop=last)
                if last:
                    # one contiguous [4, WIN] move+add covers up to 4 windows
                    gcol = slice((wi // 4) * WIN, (wi // 4) * WIN + WIN)
                    nc.vector.tensor_add(oacc[0:4, gcol], o_ps[:],
                                         oft[0:4, gcol])

            nc.scalar.dma_start(out_d[:, cb:cb + ncols], oacc[:])

        # software pipeline: block k's gathers issue before block k-1's
        # compute so the in-order Pool engine never parks a gather behind
        # output-move TTs that depend on late compute
        pending = None
        for _rep in range(repeat):
            for blk in blocks:
                state = emit_loads(blk)
                if do_compute and pending is not None:
                    emit_compute(pending)
                pending = state
        if do_compute and pending is not None:
            emit_compute(pending)

    nc.finalize()
    if nq > 1:
        # Align each gather's SWDGE queue with the DMASW sem lane Tile
        # assigned it (lane = scheduled position % 8, lanes are bound to
        # queue lane % 4 in ucode). Post-patching after scheduling beats
        # serializing the gathers to pin the order up front.
        i = 0
        for b in nc.m.functions[0].blocks:
            for ins in b.instructions:
                if type(ins).__name__ == "InstDMAGatherAnt":
                    ins.queue_num = (i % 8) % nq
                    i += 1
    _BUILD_CACHE[key] = nc
    return nc


# ---------------------------------------------------------------- entry points

def prepare(x_student, x_item, edge_label_index, edge_feat, offset, W1, b1, W2, b2):
    """Host prep + program build. Returns (nc, in_maps, metas)."""
    src = np.asarray(edge_label_index[0], np.int64)
    dst = np.asarray(edge_label_index[1], np.int64)
    ef = np.asarray(edge_feat, np.float32)

    w1_bf = np.asarray(W1, np.float32).astype(BF16)
    w2_bf = np.asarray(W2, np.float32).reshape(-1).astype(BF16)
    b1_f = np.asarray(b1, np.float32).reshape(-1)
    b2_f = float(np.asarray(b2, np.float32).reshape(-1)[0])

    # per-edge streamed constant: offset[dst] + b2 - sum(w2)  (the -sum
    # folds the "+1" of elu+1 out of the device-side accumulation)
    off = np.asarray(offset, np.float32).reshape(-1)
    off_edge = off[dst] + (b2_f - float(w2_bf.astype(np.float32).sum()))

    blocks, e_tot, cols_tot, perm, per_core = _prep_cores(src, dst, ef, off_edge)

    xs_bf = np.asarray(x_student, np.float32).astype(BF16)
    xi_bf = np.asarray(x_item, np.float32).astype(BF16)

    wpack = np.zeros((2 * IN_CH + EDGE_DIM + 1, DEC_CH), BF16)
    wpack[0:128] = w1_bf[0:128]
    wpack[128:192] = w1_bf[128:192]
    wpack[192] = b1_f.astype(BF16)          # b1 row (rhs ones row hits it)
    wpack[193:321] = w1_bf[192:320]

    global _LAST_BLOCKS
    _LAST_BLOCKS = (blocks, e_tot, cols_tot)
    nc = _build(tuple(blocks), e_tot, cols_tot)
    in_maps = []
    for c in range(N_CORES):
        pc = per_core[c]
        in_maps.append({
            "xs": xs_bf, "xi": xi_bf,
            "idx2": pc["idx2"],
            "efx": pc["efx"], "offx": pc["offx"],
            "wpack": wpack, "w2": w2_bf,
        })
    metas = [(pc["pos"], perm) for pc in per_core]
    return nc, in_maps, metas


def unshard(results, metas):
    out = np.empty((N_EDGES, 1), np.float32)
    for c in range(N_CORES):
        pos, perm = metas[c]
        valid = pos >= 0
        flat = results[c]["out"].reshape(-1)
        part = np.empty(E_PER, np.float32)
        part[pos[valid]] = flat[perm[valid]]
        out[c * E_PER:(c + 1) * E_PER, 0] = part
    return out


def kernel(x_student, x_item, edge_label_index, edge_feat, offset, W1, b1, W2, b2):
    nc, in_maps, metas = prepare(x_student, x_item, edge_label_index, edge_feat,
                                 offset, W1, b1, W2, b2)
    res = run_bass_kernel_spmd(nc, in_maps, core_ids=list(range(N_CORES)))
    return unshard(res.results, metas)

